# revision 70
# baseline (speedup 1.0000x reference)
"""Multi-head attention (B=4, S=2048, H=1024, NH=16) on 8 TRN2 NeuronCores.

Sharding: data-parallel over batch (4) x tensor-parallel over heads (2 groups
of 8 heads). Core c handles batch c//2, head-group c%2 (features 512*(c%2)..).
The host pre-transposes x to x^T [H, S] and W to W^T [H, DG] (bf16).

Per-core kernel (layout chosen so every matmul streams the minimum number of
moving rows and the ScalarE exp stream - the binding resource at ~266us -
never waits on layout shuffles):
  1. Projections: Q^T, K^T per head-pair in [feature, token] layout (bf16),
     V in [token, feature] with a ones column per (pair, head, kt-chunk).
     One batched 3D-AP DMA per input block; pair-0 weight slices load first
     so the first score chunk unblocks ~15us in; K projections for later
     token blocks precede all V/Q work since attention qb=0 walks every
     kt-chunk, while PV trails the exp stream by the 33-deep pt ring and
     Q-tb_i is only needed ~66us/qb later.
  2. Attention per head-pair p (2 heads = 128 features), per 512-token
     q-block, per 128-token kt-chunk:
       - two row-tiled QK^T matmuls produce S^T [128 kt, 512 q] per head,
       - one ScalarE activation does exp(S^T * 1/8 + mask) for both heads
         (PSUM -> SBUF bf16; the mask enters as the per-partition bias),
       - PV runs in the [q, d] orientation: stationary = P^T slice
         [128 kt, 128 q], moving = [V | ones] [128 kt, 65]; each matmul
         streams only 65 output rows (vs 512 in the [d, q] orientation),
         accumulating ctx[q, d] + sumexp[q] over the 16 kt-chunks. The four
         q-chunk regions share one PSUM bank per head, so only the bank's
         first matmul sets start=True (start pends the whole 2KB zero
         region; sibling regions accumulate onto pending-zero bytes).
  3. ctx + sumexp stage through SBUF and DMA out in [token, feature] order;
     the host only normalizes (ctx / sumexp) and concatenates.
A short warm-up matmul run keeps the PE p-state at full clock through the
initial DMA era.
"""

from contextlib import ExitStack

import numpy as np

import concourse.mybir as mybir
import concourse.tile as tile
from concourse import bacc
from concourse.bass_utils import run_bass_kernel_spmd

B, S, H, NH, HD = 4, 2048, 1024, 16, 64
NCORES = 8
DP, TP = 4, 2            # batch-parallel x head-group-parallel
HG = NH // TP            # 8 heads per core
DG = HG * HD             # 512 features per core
NPAIR = HG // 2          # 4 head pairs (128 features each)
CCH = H // 128           # 8 contraction chunks for projections
TB = S // 512            # 4 token blocks of 512
TCH = S // 128           # 16 token chunks of 128
QB = S // 512            # 4 q-blocks of 512
F32 = mybir.dt.float32
F32R = mybir.dt.float32r
BF16 = mybir.dt.bfloat16

_CACHED = None
LAST_RESULTS = None   # BassKernelResults of the most recent run (for test.py)
TRACE = False         # set True (or BASS_KERNEL_TRACE=1) to profile the run


def _build_core_program(repeat=1):
    nc = bacc.Bacc(
        "TRN2", target_bir_lowering=False, debug=False, enable_asserts=False
    )

    xqT = nc.declare_dram_parameter("xqT", [H, S], BF16, isOutput=False)
    xkT = nc.declare_dram_parameter("xkT", [H, S], BF16, isOutput=False)
    xvT = nc.declare_dram_parameter("xvT", [H, S], BF16, isOutput=False)
    wqT = nc.declare_dram_parameter("wqT", [H, DG], BF16, isOutput=False)
    wkT = nc.declare_dram_parameter("wkT", [H, DG], BF16, isOutput=False)
    wvT = nc.declare_dram_parameter("wvT", [H, DG], BF16, isOutput=False)
    # bq | bk | mask packed as one [128, NPAIR+NPAIR+TCH] f32 tensor
    cst = nc.declare_dram_parameter("cst", [128, 2 * NPAIR + TCH], F32,
                                    isOutput=False)
    bv = nc.declare_dram_parameter("bv", [128, DG], BF16, isOutput=False)
    # rows: ((pair*2 + head)*QB + qb)*128 + q_local; cols: qc*65 + (d | sumexp)
    out = nc.declare_dram_parameter("out", [NPAIR * 2 * QB * 128, 260], F32,
                                    isOutput=True)

    with tile.TileContext(nc) as tc:
        for _ in range(repeat):
            _emit(tc, nc, xqT, xkT, xvT, wqT, wkT, wvT, cst, bv, out)

    nc.compile()
    return nc


def _emit(tc, nc, xqT, xkT, xvT, wqT, wkT, wvT, cst, bv, out):
    Exp = mybir.ActivationFunctionType.Exp

    pools = ExitStack()
    const = pools.enter_context(tc.tile_pool(name="const", bufs=1))
    persist = pools.enter_context(tc.tile_pool(name="persist", bufs=1))
    xpool = pools.enter_context(tc.tile_pool(name="xpool", bufs=3))
    work = pools.enter_context(tc.tile_pool(name="work", bufs=6))
    # PSUM: sc 2x[128,1024] (4 banks) + proj 2x[128,512] (2 banks)
    #       + ctxA/ctxB [128,260] (2 banks) = 8 banks exactly
    psum = pools.enter_context(tc.tile_pool(name="psum", bufs=1, space="PSUM"))

    # ---- constants / weights ----
    cst_sb = const.tile([128, 2 * NPAIR + TCH], F32, tag="cst")
    bq_sb = cst_sb[:, 0:NPAIR]
    bk_sb = cst_sb[:, NPAIR : 2 * NPAIR]
    mask_sb = cst_sb[:, 2 * NPAIR : 2 * NPAIR + TCH]
    # V bias pre-replicated across partitions by the host so the V
    # drain-copy can add it on the DVE instead of 16 PE ones-matmuls
    bv_rep = const.tile([128, DG], BF16, tag="bv_rep")

    def load_consts():
        nc.sync.dma_start(cst_sb[:], cst[:])

    def load_bv():
        nc.sync.dma_start(bv_rep[:], bv[:])

    # PE p-state warm-up: a run of dependency-free matmuls on a zeroed tile
    # keeps the PE continuously busy through the initial DMA era so the real
    # projections start at full clock (their results are never read)
    def warmup(n):
        wz = xpool.tile([128, 512], BF16, tag="warm", name="wz", bufs=1)
        nc.gpsimd.memset(wz[:], 0.0)
        for i in range(n):
            ps = psum.tile([128, 512], F32, tag="mmp", bufs=2, name="ps")
            nc.tensor.matmul(ps[:, 0:512], wz[:, 0:128], wz[:], start=True,
                             stop=True)

    # weights as [128, cch*DG]: slice (cch, pair) at cols cch*DG + 128*p;
    # one batched 3D-AP DMA per weight matrix (HWDGE overhead is serial)
    w_sb = {}
    w_dram = {"k": wkT, "q": wqT, "v": wvT}

    def _w_views(name):
        if name not in w_sb:
            w_sb[name] = const.tile(
                [128, CCH * DG], BF16, tag=f"w{name}", name=f"w{name}"
            )
        w = w_sb[name]
        wv4 = w[:].rearrange("p (c g d) -> p c g d", c=CCH, g=NPAIR)
        sv4 = w_dram[name][:].rearrange("(c p) (g d) -> p c g d", p=128,
                                        g=NPAIR)
        return wv4, sv4

    def load_w_pair0(name):
        # pair-0 slice only: 4x less data ahead of the first Q/K unit
        wv4, sv4 = _w_views(name)
        nc.sync.dma_start(wv4[:, :, 0, :], sv4[:, :, 0, :])

    def load_w_rest(name):
        wv4, sv4 = _w_views(name)
        nc.sync.dma_start(wv4[:, :, 1:, :], sv4[:, :, 1:, :])

    def load_w(name):
        if name in w_sb:
            return w_sb[name]
        _w_views(name)
        nc.sync.dma_start(
            w_sb[name][:].rearrange("p (c d) -> p c d", c=CCH),
            w_dram[name][:].rearrange("(c p) d -> p c d", p=128),
        )
        return w_sb[name]

    # ---- persistent activations ----
    # Q^T, K^T per pair: [128 features, S tokens] (bf16: the extra rounding
    # costs ~0.2% relative error, well within budget, and halves SBUF)
    qt_sb = [
        persist.tile([128, S], BF16, tag=f"qt{p}", name=f"qt{p}")
        for p in range(NPAIR)
    ]
    kt_sb = [
        persist.tile([128, S], BF16, tag=f"kt{p}", name=f"kt{p}")
        for p in range(NPAIR)
    ]
    # V with a ones column per (pair, head, kt-chunk): col layout
    # p*(TCH*2*65) + (c*2+h)*65 + d, d in 0..64 where col 64 is ones
    v_sb = persist.tile([128, NPAIR * TCH * 2 * 65], BF16, tag="v")
    v_r = v_sb[:].rearrange("t (p c2 d) -> t p c2 d", p=NPAIR, c2=TCH * 2)
    nc.gpsimd.memset(v_r[:, :, :, 64:65], 1.0)

    # resident x_q^T [128, cch*S] bf16 (32KB/partition) so Q projection for
    # one pair needs no DMA and can interleave with attention
    xq_res = persist.tile([128, CCH * S], BF16, tag="xq_res")
    xq_res_v = xq_res[:].rearrange("p (c t) -> p c t", c=CCH)

    def load_xq_res_tb(tb):
        t_sl = slice(512 * tb, 512 * (tb + 1))
        nc.sync.dma_start(
            xq_res_v[:, :, t_sl],
            xqT[:].rearrange("(c p) t -> p c t", p=128)[:, :, t_sl],
        )

    # one staged x tile [128, cch*512] per (matrix, token block), one DMA each
    def load_x_stage(xT, key, tb, bufs=2):
        xt = xpool.tile(
            [128, CCH * 512], BF16, tag=f"x{key}", name=f"x{key}", bufs=bufs
        )
        nc.sync.dma_start(
            xt[:].rearrange("p (c t) -> p c t", c=CCH),
            xT[:].rearrange("(c p) t -> p c t", p=128)[
                :, :, 512 * tb : 512 * (tb + 1)
            ],
        )
        return xt

    # ---- projection building blocks ----
    def qk_proj_tb(xt, wkey, dst, bias_sb, prs, tb):
        # [feature, token] output for the given pairs, one token block
        for p in prs:
            ps = psum.tile([128, 512], F32, tag="mmp", bufs=2, name="ps")
            for c in range(CCH):
                nc.tensor.matmul(
                    ps[:, 0:512],
                    (w_sb[wkey][:, c * DG + 128 * p : c * DG + 128 * (p + 1)]),
                    (xt[:, 512 * c : 512 * (c + 1)]),
                    start=(c == 0),
                    stop=(c == CCH - 1),
                )
            nc.vector.tensor_scalar_add(
                dst[p][:, 512 * tb : 512 * (tb + 1)],
                ps[:, 0:512],
                bias_sb[:, p : p + 1],
            )

    def v_proj_tb(xt, tb):
        # V[token, feature], all pairs, one token block of x_v^T
        for j in range(4):
            c = 4 * tb + j
            ps = psum.tile([128, 512], F32, tag="mmp", bufs=2, name="ps")
            for cc in range(CCH):
                nc.tensor.matmul(
                    ps[:, 0:512],
                    (xt[:, 512 * cc + 128 * j : 512 * cc + 128 * (j + 1)]),
                    (w_sb["v"][:, cc * DG : (cc + 1) * DG]),
                    start=(cc == 0),
                    stop=(cc == CCH - 1),
                )
            nc.vector.tensor_add(
                v_r[:, :, 2 * c : 2 * c + 2, 0:64],
                ps[:, 0:512].rearrange("t (p h d) -> t p h d", p=NPAIR, h=2),
                bv_rep[:].rearrange("t (p h d) -> t p h d", p=NPAIR, h=2),
            )

    def q_proj_tb(p, tb):
        load_w("q")
        ps = psum.tile([128, 512], F32, tag="mmp", bufs=2, name="ps")
        for c in range(CCH):
            nc.tensor.matmul(
                ps[:, 0:512],
                (w_sb["q"][:, c * DG + 128 * p : c * DG + 128 * (p + 1)]),
                (xq_res[:, c * S + 512 * tb : c * S + 512 * (tb + 1)]),
                start=(c == 0),
                stop=(c == CCH - 1),
            )
        nc.vector.tensor_scalar_add(
            qt_sb[p][:, 512 * tb : 512 * (tb + 1)],
            ps[:, 0:512],
            bq_sb[:, p : p + 1],
        )

    # ---- attention for one (pair, q-block) ----
    def attention_block(p, qb):
        q_sl = slice(512 * qb, 512 * (qb + 1))
        # ctx accumulators in [q, d] orientation: one bank per head holding
        # 4 q-chunks x (64 ctx + 1 sumexp) columns
        ctx_ps = [
            psum.tile([128, 260], F32, tag=f"ctx{h}", name=f"ctx{h}")
            for h in range(2)
        ]
        for c in range(TCH):
            kt_sl = slice(128 * c, 128 * (c + 1))
            sc = psum.tile([128, 1024], F32, tag="mm", bufs=2)
            for h in (0, 1):
                hp = slice(64 * h, 64 * (h + 1))
                nc.tensor.matmul(
                    sc[:, 512 * h : 512 * (h + 1)],
                    (kt_sb[p][hp, kt_sl]),
                    (qt_sb[p][hp, q_sl]),
                    start=True,
                    stop=True,
                )
            pt = work.tile([128, 1024], BF16, tag="pt", bufs=32)
            nc.scalar.activation(
                pt[:], sc[:], Exp, bias=mask_sb[:, c : c + 1], scale=0.125
            )
            for h in range(2):
                for qc in range(4):
                    # start=True pends-to-zero the WHOLE 2KB psum bank
                    # (ZERO_REGION_SIZE), so only the bank's first matmul may
                    # set it; the other qc regions accumulate onto
                    # pending-zero bytes, which read as zero.
                    nc.tensor.matmul(
                        ctx_ps[h][:, 65 * qc : 65 * qc + 65],
                        (pt[:, 512 * h + 128 * qc : 512 * h + 128 * (qc + 1)]),
                        (v_r[:, p, 2 * c + h, :]),
                        start=(c == 0 and qc == 0),
                        stop=(c == TCH - 1),
                        skip_group_check=True,
                    )
        # ctx + sumexp: PSUM -> SBUF staging (DVE) -> DRAM; host normalizes
        for h in range(2):
            cs = work.tile([128, 260], F32, tag="cs", name="cs", bufs=3)
            nc.vector.tensor_copy(out=cs[:], in_=ctx_ps[h][:])
            r0 = ((p * 2 + h) * QB + qb) * 128
            nc.sync.dma_start(out[r0 : r0 + 128, :], cs[:])

    # ---- phase order ----
    # DMA + emission order front-loads what attention (p0, qb=0) chunk 0
    # needs: W_k, x_k tb0, consts, W_q, x_q tb0, then K/Q projections
    # pair-interleaved so pair 0's score chain unblocks first. Attention is
    # emitted inside tc.high_priority() so the scheduler treats it as
    # earliest work: each attention chunk fires the moment its K-block /
    # V-chunk / Q-block lands, and the remaining projection matmuls fill the
    # PE whenever attention is waiting on the ScalarE exp chain.
    warmup(28)
    load_w_pair0("k")
    allp = list(range(NPAIR))
    xk0 = load_x_stage(xkT, "k", 0)
    load_consts()
    load_w_pair0("q")
    load_xq_res_tb(0)
    load_w_rest("k")
    load_w_rest("q")
    # pair-interleaved K/Q for tb0 so pair 0's score chain unblocks first
    for p in allp:
        qk_proj_tb(xk0, "k", kt_sb, bk_sb, [p], 0)
        q_proj_tb(p, 0)
    # K for tb1-3 next (DMAs and projections): attention qb=0 walks all 16
    # kt-chunks, so these gate the exp stream. V only feeds PV, which trails
    # the exp stream by the pt ring; Q for tb>=1 is needed one qb-round
    # (~66us of exp) later still.
    xk_rest = [load_x_stage(xkT, "k", tb) for tb in range(1, TB)]
    for tb in range(1, TB):
        qk_proj_tb(xk_rest[tb - 1], "k", kt_sb, bk_sb, allp, tb)
    load_bv()
    load_w("v")
    xv = [load_x_stage(xvT, "v", tb) for tb in range(TB)]
    for tb in range(TB):
        v_proj_tb(xv[tb], tb)
    for tb in range(1, TB):
        load_xq_res_tb(tb)
        for p in allp:
            q_proj_tb(p, tb)
    for p, qb in [(p, qb) for qb in range(QB) for p in allp]:
        with tc.high_priority():
            attention_block(p, qb)

    pools.close()


def make_in_maps(x_q, x_k, x_v, att_mask, W_q, b_q, W_k, b_k, W_v, b_v):
    import ml_dtypes

    f = np.float32
    bf = ml_dtypes.bfloat16
    x_q, x_k, x_v = (np.asarray(a, f) for a in (x_q, x_k, x_v))
    att_mask = np.asarray(att_mask, f)
    W_q, W_k, W_v = (np.asarray(a, f) for a in (W_q, W_k, W_v))
    b_q, b_k, b_v = (np.asarray(a, f) for a in (b_q, b_k, b_v))

    in_maps = []
    for core in range(NCORES):
        b, g = divmod(core, TP)
        fsl = slice(DG * g, DG * (g + 1))
        in_maps.append(
            {
                "xqT": np.ascontiguousarray(x_q[b].T.astype(bf)),
                "xkT": np.ascontiguousarray(x_k[b].T.astype(bf)),
                "xvT": np.ascontiguousarray(x_v[b].T.astype(bf)),
                "wqT": np.ascontiguousarray(W_q[fsl, :].T.astype(bf)),
                "wkT": np.ascontiguousarray(W_k[fsl, :].T.astype(bf)),
                "wvT": np.ascontiguousarray(W_v[fsl, :].T.astype(bf)),
                "cst": np.ascontiguousarray(
                    np.concatenate(
                        [
                            b_q[fsl].reshape(NPAIR, 128).T,
                            b_k[fsl].reshape(NPAIR, 128).T,
                            att_mask[b, 0, 0].reshape(TCH, 128).T,
                        ],
                        axis=1,
                    )
                ),
                "bv": np.ascontiguousarray(
                    np.broadcast_to(b_v[fsl].reshape(1, DG), (128, DG))
                ).astype(bf),
            }
        )
    return in_maps


def kernel(x_q, x_k, x_v, att_mask, W_q, b_q, W_k, b_k, W_v, b_v):
    global _CACHED
    if _CACHED is None:
        _CACHED = _build_core_program()
    nc = _CACHED

    in_maps = make_in_maps(
        x_q, x_k, x_v, att_mask, W_q, b_q, W_k, b_k, W_v, b_v
    )

    import os

    global LAST_RESULTS
    trace = TRACE or os.environ.get("BASS_KERNEL_TRACE", "") == "1"
    try:
        res = run_bass_kernel_spmd(nc, in_maps, list(range(NCORES)), trace=trace)
    except Exception:
        if not trace:
            raise
        # profiling hook unavailable (e.g. trimmed container) - run untraced
        res = run_bass_kernel_spmd(nc, in_maps, list(range(NCORES)))
    LAST_RESULTS = res

    # out rows are ((pair*2+h)*QB + qb)*128 + q_local, cols qc*65 + (d|sumexp);
    # normalize by the sumexp column and reassemble on the host
    full = np.empty((B, S, H), np.float32)
    for core in range(NCORES):
        b, g = divmod(core, TP)
        r = res.results[core]["out"].reshape(NPAIR, 2, QB, 128, 4, 65)
        ctx = r[..., 0:64] / r[..., 64:65]          # [p, h, qb, i, qc, d]
        # q = qb*512 + qc*128 + i ; feature = (p*2+h)*64 + d
        full[b, :, DG * g : DG * (g + 1)] = (
            ctx.transpose(2, 4, 3, 0, 1, 5).reshape(S, DG)
        )
    return full
